# revision 13
# baseline (speedup 1.0000x reference)
"""Trainium2 Bass kernel: RK4 neural-ODE solver (nn_DiffeqSolver).

Reference semantics (see problem): MLP f(h) = tanh(tanh(h@W1+b1)@W2+b2)@W3+b3,
integrated with RK4 over a time grid t (199 steps), returning all states
[B, T, H].

Strategy
--------
- Data-parallel: batch B=4096 split across 8 NeuronCores (512 rows each).
- Feature-major on-chip layout: h is [H=64 (partitions), batch (free)], so each
  MLP matmul is a single TensorE matmul with the (tiny) weight as the
  stationary operand and batch as the moving free dim.
- Each core's 512 rows are split into 2 independent 256-wide "streams" whose
  199-step chains never interact -> Tile pipelines them across TensorE (matmul,
  float32r fast path), ScalarE (tanh+bias) and DVE (RK4 axpy/combine).
- b3 is folded into per-step biased states: h_b05 = h + 0.5*dt*b3 and
  h_bd = h + dt*b3 (precomputed host tables outer(b3, dts)), so the last
  matmul's output g = f(h) - b3 can be consumed directly from PSUM.
- The RK4 sum k1+2k2+2k3+k4 (in g-space) is accumulated in a single PSUM bank
  by 4 matmuls using stationary W3 / 2*W3; combine is ONE DVE op:
  h_next = (dt/6)*S + h_bd.
- Per-step dt immediates are baked at build time from the actual `t` input,
  so non-uniform grids work.
- Output is written time-major [T, H, 512] per core (perfectly contiguous DMA)
  and transposed to [512, T, H] on the host.
"""

import os
import sys

import numpy as np

for _p in ("/opt/trn_rl_repo", "/root/.axon_site/_ro/trn_rl_repo"):
    if os.path.isdir(_p) and _p not in sys.path:
        sys.path.insert(0, _p)

import concourse.bass as bass
import concourse.mybir as mybir
import concourse.tile as tile
from concourse.bass_utils import run_bass_kernel_spmd

F32 = mybir.dt.float32
F32R = mybir.dt.float32r
AF = mybir.ActivationFunctionType
OP = mybir.AluOpType

N_CORES = 8
H = 64    # state dim
HT = 100  # hidden dim
NSTREAM = 2


def _legalize_waits(nc: bass.Bass, max_waits: int = 1) -> int:
    """This container's walrus encodes at most ONE sync-wait per instruction
    (hardware EVENTS struct); Tile can attach several. Hoist excess waits onto
    injected same-engine NoOps placed immediately before the instruction —
    engine streams execute in order, so semantics are preserved."""
    n_new = 0
    for fn in nc.m.functions:
        for bb in fn.blocks:
            new_list = []
            changed = False
            for ins in bb.instructions:
                si = ins.sync_info
                waits = list(si.on_wait) if si and si.on_wait else []
                if len(waits) > max_waits:
                    keep = waits[-max_waits:]
                    for w in waits[:-max_waits]:
                        nop = mybir.InstNoOp(name=f"I-waitsplit-{n_new}")
                        n_new += 1
                        nop.engine = ins.engine
                        nop.sync_info = mybir.SyncInfo(on_wait=[w], on_update=[])
                        new_list.append(nop)
                    ins.sync_info = mybir.SyncInfo(
                        on_wait=keep,
                        on_update=list(si.on_update) if si.on_update else [],
                    )
                    changed = True
                new_list.append(ins)
            if changed:
                bb.instructions = new_list
    return n_new


def build_program(dts: np.ndarray, b_local: int, mm_fast: bool = True,
                  reps: int = 1) -> bass.Bass:
    """Build the per-core Bass program. Same program runs on all 8 cores
    (pure data parallel, no collectives). reps>1 repeats the whole
    integration (identical output) — used only for wall-clock timing."""
    n_steps = len(dts)
    T = n_steps + 1
    cw = b_local // NSTREAM  # stream width (256)

    nc = bass.Bass(trn_type="TRN2", target_bir_lowering=False, debug=False)

    h0t = nc.dram_tensor("h0t", [H, b_local], F32, kind="ExternalInput").ap()
    w1 = nc.dram_tensor("w1", [H, HT], F32, kind="ExternalInput").ap()
    w2 = nc.dram_tensor("w2", [HT, HT], F32, kind="ExternalInput").ap()
    w3 = nc.dram_tensor("w3", [HT, H], F32, kind="ExternalInput").ap()
    w3x2 = nc.dram_tensor("w3x2", [HT, H], F32, kind="ExternalInput").ap()
    b1d = nc.dram_tensor("b1c", [HT, 1], F32, kind="ExternalInput").ap()
    b2d = nc.dram_tensor("b2c", [HT, 1], F32, kind="ExternalInput").ap()
    tab05 = nc.dram_tensor("tab05", [H, n_steps], F32, kind="ExternalInput").ap()
    tabd = nc.dram_tensor("tabd", [H, n_steps], F32, kind="ExternalInput").ap()
    out = nc.dram_tensor("out", [T, H, b_local], F32, kind="ExternalOutput").ap()

    MMDT = F32R if mm_fast else F32  # dtype of matmul operand tiles

    with tile.TileContext(nc) as tc:
        with (
            tc.tile_pool(name="const", bufs=1) as cp,
            tc.tile_pool(name="sb", bufs=1) as sb,
            tc.tile_pool(name="ps", bufs=1, space="PSUM") as ps,
        ):
            W1t = cp.tile([H, HT], MMDT, tag="w1")
            W2t = cp.tile([HT, HT], MMDT, tag="w2")
            W3t = cp.tile([HT, H], MMDT, tag="w3")
            W3x2t = cp.tile([HT, H], MMDT, tag="w3x2")
            b1t = cp.tile([HT, 1], F32, tag="b1")
            b2t = cp.tile([HT, 1], F32, tag="b2")
            t05 = cp.tile([H, n_steps], F32, tag="t05")
            tdt = cp.tile([H, n_steps], F32, tag="tdt")
            for dst, src in (
                (b1t, b1d), (b2t, b2d), (t05, tab05), (tdt, tabd),
            ):
                nc.sync.dma_start(out=dst[:], in_=src)
            # weights: DMA to fp32 staging, then DVE-convert ("round") into
            # the matmul dtype — walrus requires f32r matmul operands to be
            # produced by a rounding-capable compute op, not raw DMA.
            for dst, src, shp, wtag in (
                (W1t, w1, [H, HT], "w1s"), (W2t, w2, [HT, HT], "w2s"),
                (W3t, w3, [HT, H], "w3s"), (W3x2t, w3x2, [HT, H], "w4s"),
            ):
                if mm_fast:
                    stage = sb.tile(shp, F32, tag=wtag, name="wstage")
                    nc.sync.dma_start(out=stage[:], in_=src)
                    nc.vector.tensor_copy(dst[:], stage[:])
                else:
                    nc.sync.dma_start(out=dst[:], in_=src)

            for _rep in range(reps):
                # initial states per stream + t=0 output rows
                h_cur = []
                for s in range(NSTREAM):
                    c0 = s * cw
                    h0s = sb.tile([H, cw], F32, tag=f"h{s}", bufs=3)
                    nc.sync.dma_start(out=h0s[:], in_=h0t[:, c0:c0 + cw])
                    nc.sync.dma_start(out=out[0, :, c0:c0 + cw], in_=h0s[:])
                    h_cur.append(h0s)

                for i in range(n_steps):
                  dt = np.float32(dts[i])
                  c_half = float(np.float32(0.5) * dt)
                  c_full = float(dt)
                  c_sixth = float(dt / np.float32(6.0))
                  for s in range(NSTREAM):
                    c0 = s * cw
                    h = h_cur[s]

                    # biased base states (b3 folded): h + 0.5*dt*b3, h + dt*b3
                    hb05 = sb.tile([H, cw], F32, tag=f"hb05_{s}", bufs=2)
                    nc.vector.tensor_scalar_add(hb05[:], h[:], t05[:, i:i + 1])
                    hbd = sb.tile([H, cw], F32, tag=f"hbd_{s}", bufs=2)
                    nc.vector.tensor_scalar_add(hbd[:], h[:], tdt[:, i:i + 1])

                    if mm_fast:
                        # rounded copy of the fp32 state for eval-1's matmul
                        hin = sb.tile([H, cw], MMDT, tag=f"hr{s}", bufs=2)
                        nc.vector.tensor_copy(hin[:], h[:])
                    else:
                        hin = h

                    Sp = ps.tile([H, cw], F32, tag=f"S{s}", bufs=1)

                    for e in range(4):
                        # z1 = tanh(W1.T @ hin + b1)
                        z1p = ps.tile([HT, cw], F32, tag=f"zg{s}", bufs=3)
                        nc.tensor.matmul(z1p[:], W1t[:], hin[:],
                                         start=True, stop=True)
                        z1s = sb.tile([HT, cw], MMDT, tag=f"z{s}", bufs=2)
                        nc.scalar.activation(z1s[:], z1p[:], AF.Tanh, bias=b1t[:])
                        # z2 = tanh(W2.T @ z1 + b2)
                        z2p = ps.tile([HT, cw], F32, tag=f"zg{s}", bufs=3)
                        nc.tensor.matmul(z2p[:], W2t[:], z1s[:],
                                         start=True, stop=True)
                        z2s = sb.tile([HT, cw], MMDT, tag=f"z{s}", bufs=2)
                        nc.scalar.activation(z2s[:], z2p[:], AF.Tanh, bias=b2t[:])
                        # g_e = W3.T @ z2 (= k_e - b3); accumulate RK4 sum in Sp
                        # with weights 1,2,2,1 via stationary W3 / 2*W3.
                        w_acc = W3t if e in (0, 3) else W3x2t
                        nc.tensor.matmul(Sp[:], w_acc[:], z2s[:],
                                         start=(e == 0), stop=(e == 3))
                        if e < 3:
                            # g_e also to its own bank, to build eval e+1 input
                            ge = ps.tile([HT, cw], F32, tag=f"zg{s}", bufs=3)
                            nc.tensor.matmul(ge[:H, :], W3t[:], z2s[:],
                                             start=True, stop=True)
                            # next eval input: base + c*g_e
                            c = c_half if e < 2 else c_full
                            base = hb05 if e < 2 else hbd
                            hin = sb.tile([H, cw], MMDT, tag=f"hin{s}", bufs=2)
                            nc.vector.scalar_tensor_tensor(
                                hin[:], ge[:H, :], c, base[:], OP.mult, OP.add)

                    # h_next = h + dt*b3 + (dt/6) * S
                    hn = sb.tile([H, cw], F32, tag=f"h{s}", bufs=3)
                    nc.vector.scalar_tensor_tensor(
                        hn[:], Sp[:], c_sixth, hbd[:], OP.mult, OP.add)
                    nc.sync.dma_start(out=out[i + 1, :, c0:c0 + cw], in_=hn[:])
                    h_cur[s] = hn
    return nc


def kernel(h0, t, W1, b1, W2, b2, W3, b3):
    h0 = np.ascontiguousarray(np.asarray(h0, dtype=np.float32))
    t = np.asarray(t, dtype=np.float32)
    W1 = np.ascontiguousarray(np.asarray(W1, dtype=np.float32))
    b1 = np.asarray(b1, dtype=np.float32)
    W2 = np.ascontiguousarray(np.asarray(W2, dtype=np.float32))
    b2 = np.asarray(b2, dtype=np.float32)
    W3 = np.ascontiguousarray(np.asarray(W3, dtype=np.float32))
    b3 = np.asarray(b3, dtype=np.float32)

    B = h0.shape[0]
    T = t.shape[0]
    b_local = B // N_CORES

    dts = (t[1:] - t[:-1]).astype(np.float32)
    nc = build_program(dts, b_local, mm_fast=MM_FAST)
    _legalize_waits(nc)

    tab05 = np.ascontiguousarray(np.outer(b3, np.float32(0.5) * dts).astype(np.float32))
    tabd = np.ascontiguousarray(np.outer(b3, dts).astype(np.float32))
    w3x2 = (np.float32(2.0) * W3).astype(np.float32)

    common = {
        "w1": W1,
        "w2": W2,
        "w3": W3,
        "w3x2": w3x2,
        "b1c": np.ascontiguousarray(b1.reshape(HT, 1)),
        "b2c": np.ascontiguousarray(b2.reshape(HT, 1)),
        "tab05": tab05,
        "tabd": tabd,
    }
    in_maps = []
    for c in range(N_CORES):
        h0c = np.ascontiguousarray(h0[c * b_local:(c + 1) * b_local].T)
        in_maps.append({**common, "h0t": h0c})

    trace = bool(os.environ.get("KERNEL_TRACE"))
    res = run_bass_kernel_spmd(nc, in_maps, list(range(N_CORES)), trace=trace)
    global LAST_RESULTS
    LAST_RESULTS = res

    full = np.empty((B, T, h0.shape[1]), np.float32)
    for c in range(N_CORES):
        # [T, H, b_local] -> [b_local, T, H]
        full[c * b_local:(c + 1) * b_local] = res.results[c]["out"].transpose(2, 0, 1)
    return full


MM_FAST = True  # float32r matmul fast path (1 cyc/row at N>=256)
LAST_RESULTS = None  # BassKernelResults of the most recent run (for test.py)


# revision 15
# speedup vs baseline: 6.2356x; 6.2356x over previous
"""Trainium2 Bass kernel: RK4 neural-ODE solver (nn_DiffeqSolver).

Reference semantics (see problem): MLP f(h) = tanh(tanh(h@W1+b1)@W2+b2)@W3+b3,
integrated with RK4 over a time grid t (199 steps), returning all states
[B, T, H].

Strategy
--------
- Data-parallel: batch B=4096 split across 8 NeuronCores (512 rows each).
- Feature-major on-chip layout: h is [H=64 (partitions), batch (free)], so each
  MLP matmul is a single TensorE matmul with the (tiny) weight as the
  stationary operand and batch as the moving free dim.
- Each core's 512 rows are split into 2 independent 256-wide "streams" whose
  199-step chains never interact -> Tile pipelines them across TensorE (matmul,
  float32r fast path), ScalarE (tanh+bias) and DVE (RK4 axpy/combine).
- b3 is folded into per-step biased states: h_b05 = h + 0.5*dt*b3 and
  h_bd = h + dt*b3 (precomputed host tables outer(b3, dts)), so the last
  matmul's output g = f(h) - b3 can be consumed directly from PSUM.
- The RK4 sum k1+2k2+2k3+k4 (in g-space) is accumulated in a single PSUM bank
  by 4 matmuls using stationary W3 / 2*W3; combine is ONE DVE op:
  h_next = (dt/6)*S + h_bd.
- Per-step dt immediates are baked at build time from the actual `t` input,
  so non-uniform grids work.
- Output is written time-major [T, H, 512] per core (perfectly contiguous DMA)
  and transposed to [512, T, H] on the host.
"""

import os
import sys

import numpy as np

for _p in ("/opt/trn_rl_repo", "/root/.axon_site/_ro/trn_rl_repo"):
    if os.path.isdir(_p) and _p not in sys.path:
        sys.path.insert(0, _p)

import concourse.bass as bass
import concourse.mybir as mybir
import concourse.tile as tile
from concourse.bass_utils import run_bass_kernel_spmd

F32 = mybir.dt.float32
F32R = mybir.dt.float32r
AF = mybir.ActivationFunctionType
OP = mybir.AluOpType

N_CORES = 8
H = 64    # state dim
HT = 100  # hidden dim
NSTREAM = 2


def _legalize_waits(nc: bass.Bass, max_waits: int = 1) -> int:
    """This container's walrus encodes at most ONE sync-wait per instruction
    (hardware EVENTS struct); Tile can attach several. Hoist excess waits onto
    injected same-engine NoOps placed immediately before the instruction —
    engine streams execute in order, so semantics are preserved."""
    n_new = 0
    for fn in nc.m.functions:
        for bb in fn.blocks:
            new_list = []
            changed = False
            for ins in bb.instructions:
                si = ins.sync_info
                waits = list(si.on_wait) if si and si.on_wait else []
                if len(waits) > max_waits:
                    keep = waits[-max_waits:]
                    for w in waits[:-max_waits]:
                        nop = mybir.InstNoOp(name=f"I-waitsplit-{n_new}")
                        n_new += 1
                        nop.engine = ins.engine
                        nop.sync_info = mybir.SyncInfo(on_wait=[w], on_update=[])
                        new_list.append(nop)
                    ins.sync_info = mybir.SyncInfo(
                        on_wait=keep,
                        on_update=list(si.on_update) if si.on_update else [],
                    )
                    changed = True
                new_list.append(ins)
            if changed:
                bb.instructions = new_list
    return n_new


def build_program(dts: np.ndarray, b_local: int, mm_fast: bool = True,
                  reps: int = 1, timing_mode: bool = False) -> bass.Bass:
    """Build the per-core Bass program. Same program runs on all 8 cores
    (pure data parallel, no collectives). reps>1 repeats the whole
    integration (identical output) — used only for wall-clock timing.
    timing_mode shrinks the output buffer to [2,H,b_local] (every step
    overwrites row 1) so wall-clock isn't dominated by output transfer."""
    n_steps = len(dts)
    T = 2 if timing_mode else n_steps + 1
    cw = b_local // NSTREAM  # stream width (256)

    nc = bass.Bass(trn_type="TRN2", target_bir_lowering=False, debug=False)

    h0t = nc.dram_tensor("h0t", [H, b_local], F32, kind="ExternalInput").ap()
    w1 = nc.dram_tensor("w1", [H, HT], F32, kind="ExternalInput").ap()
    w2 = nc.dram_tensor("w2", [HT, HT], F32, kind="ExternalInput").ap()
    w3 = nc.dram_tensor("w3", [HT, H], F32, kind="ExternalInput").ap()
    w3x2 = nc.dram_tensor("w3x2", [HT, H], F32, kind="ExternalInput").ap()
    b1d = nc.dram_tensor("b1c", [HT, 1], F32, kind="ExternalInput").ap()
    b2d = nc.dram_tensor("b2c", [HT, 1], F32, kind="ExternalInput").ap()
    tab05 = nc.dram_tensor("tab05", [H, n_steps], F32, kind="ExternalInput").ap()
    tabd = nc.dram_tensor("tabd", [H, n_steps], F32, kind="ExternalInput").ap()
    out = nc.dram_tensor("out", [T, H, b_local], F32, kind="ExternalOutput").ap()

    MMDT = F32R if mm_fast else F32  # dtype of matmul operand tiles

    with tile.TileContext(nc) as tc:
        with (
            tc.tile_pool(name="const", bufs=1) as cp,
            tc.tile_pool(name="sb", bufs=1) as sb,
            tc.tile_pool(name="ps", bufs=1, space="PSUM") as ps,
        ):
            W1t = cp.tile([H, HT], MMDT, tag="w1")
            W2t = cp.tile([HT, HT], MMDT, tag="w2")
            W3t = cp.tile([HT, H], MMDT, tag="w3")
            W3x2t = cp.tile([HT, H], MMDT, tag="w3x2")
            b1t = cp.tile([HT, 1], F32, tag="b1")
            b2t = cp.tile([HT, 1], F32, tag="b2")
            t05 = cp.tile([H, n_steps], F32, tag="t05")
            tdt = cp.tile([H, n_steps], F32, tag="tdt")
            for dst, src in (
                (b1t, b1d), (b2t, b2d), (t05, tab05), (tdt, tabd),
            ):
                nc.sync.dma_start(out=dst[:], in_=src)
            # weights: DMA to fp32 staging, then DVE-convert ("round") into
            # the matmul dtype — walrus requires f32r matmul operands to be
            # produced by a rounding-capable compute op, not raw DMA.
            for dst, src, shp, wtag in (
                (W1t, w1, [H, HT], "w1s"), (W2t, w2, [HT, HT], "w2s"),
                (W3t, w3, [HT, H], "w3s"), (W3x2t, w3x2, [HT, H], "w4s"),
            ):
                if mm_fast:
                    stage = sb.tile(shp, F32, tag=wtag, name="wstage")
                    nc.sync.dma_start(out=stage[:], in_=src)
                    nc.vector.tensor_copy(dst[:], stage[:])
                else:
                    nc.sync.dma_start(out=dst[:], in_=src)

            for _rep in range(reps):
                # initial states per stream + t=0 output rows
                h_cur = []
                for s in range(NSTREAM):
                    c0 = s * cw
                    h0s = sb.tile([H, cw], F32, tag=f"h{s}", bufs=3)
                    nc.sync.dma_start(out=h0s[:], in_=h0t[:, c0:c0 + cw])
                    nc.sync.dma_start(out=out[0, :, c0:c0 + cw], in_=h0s[:])
                    h_cur.append(h0s)

                for i in range(n_steps):
                  dt = np.float32(dts[i])
                  c_half = float(np.float32(0.5) * dt)
                  c_full = float(dt)
                  c_sixth = float(dt / np.float32(6.0))
                  for s in range(NSTREAM):
                    c0 = s * cw
                    h = h_cur[s]

                    # biased base states (b3 folded): h + 0.5*dt*b3, h + dt*b3
                    hb05 = sb.tile([H, cw], F32, tag=f"hb05_{s}", bufs=2)
                    nc.vector.tensor_scalar_add(hb05[:], h[:], t05[:, i:i + 1])
                    hbd = sb.tile([H, cw], F32, tag=f"hbd_{s}", bufs=2)
                    nc.vector.tensor_scalar_add(hbd[:], h[:], tdt[:, i:i + 1])

                    if mm_fast:
                        # rounded copy of the fp32 state for eval-1's matmul
                        hin = sb.tile([H, cw], MMDT, tag=f"hr{s}", bufs=2)
                        nc.vector.tensor_copy(hin[:], h[:])
                    else:
                        hin = h

                    Sp = ps.tile([H, cw], F32, tag=f"S{s}", bufs=1)

                    for e in range(4):
                        # z1 = tanh(W1.T @ hin + b1)
                        z1p = ps.tile([HT, cw], F32, tag=f"zg{s}", bufs=3)
                        nc.tensor.matmul(z1p[:], W1t[:], hin[:],
                                         start=True, stop=True)
                        z1s = sb.tile([HT, cw], MMDT, tag=f"z{s}", bufs=2)
                        nc.scalar.activation(z1s[:], z1p[:], AF.Tanh, bias=b1t[:])
                        # z2 = tanh(W2.T @ z1 + b2)
                        z2p = ps.tile([HT, cw], F32, tag=f"zg{s}", bufs=3)
                        nc.tensor.matmul(z2p[:], W2t[:], z1s[:],
                                         start=True, stop=True)
                        z2s = sb.tile([HT, cw], MMDT, tag=f"z{s}", bufs=2)
                        nc.scalar.activation(z2s[:], z2p[:], AF.Tanh, bias=b2t[:])
                        # g_e = W3.T @ z2 (= k_e - b3); accumulate RK4 sum in Sp
                        # with weights 1,2,2,1 via stationary W3 / 2*W3.
                        w_acc = W3t if e in (0, 3) else W3x2t
                        nc.tensor.matmul(Sp[:], w_acc[:], z2s[:],
                                         start=(e == 0), stop=(e == 3))
                        if e < 3:
                            # g_e also to its own bank, to build eval e+1 input
                            ge = ps.tile([HT, cw], F32, tag=f"zg{s}", bufs=3)
                            nc.tensor.matmul(ge[:H, :], W3t[:], z2s[:],
                                             start=True, stop=True)
                            # next eval input: base + c*g_e
                            c = c_half if e < 2 else c_full
                            base = hb05 if e < 2 else hbd
                            hin = sb.tile([H, cw], MMDT, tag=f"hin{s}", bufs=2)
                            nc.vector.scalar_tensor_tensor(
                                hin[:], ge[:H, :], c, base[:], OP.mult, OP.add)

                    # h_next = h + dt*b3 + (dt/6) * S
                    hn = sb.tile([H, cw], F32, tag=f"h{s}", bufs=3)
                    nc.vector.scalar_tensor_tensor(
                        hn[:], Sp[:], c_sixth, hbd[:], OP.mult, OP.add)
                    t_out = 1 if timing_mode else i + 1
                    nc.sync.dma_start(out=out[t_out, :, c0:c0 + cw], in_=hn[:])
                    h_cur[s] = hn
    return nc


def kernel(h0, t, W1, b1, W2, b2, W3, b3):
    h0 = np.ascontiguousarray(np.asarray(h0, dtype=np.float32))
    t = np.asarray(t, dtype=np.float32)
    W1 = np.ascontiguousarray(np.asarray(W1, dtype=np.float32))
    b1 = np.asarray(b1, dtype=np.float32)
    W2 = np.ascontiguousarray(np.asarray(W2, dtype=np.float32))
    b2 = np.asarray(b2, dtype=np.float32)
    W3 = np.ascontiguousarray(np.asarray(W3, dtype=np.float32))
    b3 = np.asarray(b3, dtype=np.float32)

    B = h0.shape[0]
    T = t.shape[0]
    b_local = B // N_CORES

    dts = (t[1:] - t[:-1]).astype(np.float32)
    nc = build_program(dts, b_local, mm_fast=MM_FAST)
    _legalize_waits(nc)

    tab05 = np.ascontiguousarray(np.outer(b3, np.float32(0.5) * dts).astype(np.float32))
    tabd = np.ascontiguousarray(np.outer(b3, dts).astype(np.float32))
    w3x2 = (np.float32(2.0) * W3).astype(np.float32)

    common = {
        "w1": W1,
        "w2": W2,
        "w3": W3,
        "w3x2": w3x2,
        "b1c": np.ascontiguousarray(b1.reshape(HT, 1)),
        "b2c": np.ascontiguousarray(b2.reshape(HT, 1)),
        "tab05": tab05,
        "tabd": tabd,
    }
    in_maps = []
    for c in range(N_CORES):
        h0c = np.ascontiguousarray(h0[c * b_local:(c + 1) * b_local].T)
        in_maps.append({**common, "h0t": h0c})

    trace = bool(os.environ.get("KERNEL_TRACE"))
    res = run_bass_kernel_spmd(nc, in_maps, list(range(N_CORES)), trace=trace)
    global LAST_RESULTS
    LAST_RESULTS = res

    full = np.empty((B, T, h0.shape[1]), np.float32)
    for c in range(N_CORES):
        # [T, H, b_local] -> [b_local, T, H]
        full[c * b_local:(c + 1) * b_local] = res.results[c]["out"].transpose(2, 0, 1)
    return full


MM_FAST = True  # float32r matmul fast path (1 cyc/row at N>=256)
LAST_RESULTS = None  # BassKernelResults of the most recent run (for test.py)


# revision 23
# speedup vs baseline: 7.2688x; 1.1657x over previous
"""Trainium2 Bass kernel: RK4 neural-ODE solver (nn_DiffeqSolver).

Reference semantics (see problem): MLP f(h) = tanh(tanh(h@W1+b1)@W2+b2)@W3+b3,
integrated with RK4 over a time grid t (199 steps), returning all states
[B, T, H].

Strategy
--------
- Data-parallel: batch B=4096 split across 8 NeuronCores (512 rows each).
- Feature-major on-chip layout: h is [H=64 (partitions), batch (free)], so each
  MLP matmul is a single TensorE matmul with the (tiny) weight as the
  stationary operand and batch as the moving free dim.
- Each core's 512 rows are split into 2 independent 256-wide "streams" whose
  199-step chains never interact -> Tile pipelines them across TensorE (matmul,
  float32r fast path), ScalarE (tanh+bias) and DVE (RK4 axpy/combine).
- b3 is folded into per-step biased states: h_b05 = h + 0.5*dt*b3 and
  h_bd = h + dt*b3 (precomputed host tables outer(b3, dts)), so the last
  matmul's output g = f(h) - b3 can be consumed directly from PSUM.
- The RK4 sum k1+2k2+2k3+k4 (in g-space) is accumulated in a single PSUM bank
  by 4 matmuls using stationary W3 / 2*W3; combine is ONE DVE op:
  h_next = (dt/6)*S + h_bd.
- Per-step dt immediates are baked at build time from the actual `t` input,
  so non-uniform grids work.
- Output is written time-major [T, H, 512] per core (perfectly contiguous DMA)
  and transposed to [512, T, H] on the host.
"""

import os
import sys

import numpy as np

for _p in ("/opt/trn_rl_repo", "/root/.axon_site/_ro/trn_rl_repo"):
    if os.path.isdir(_p) and _p not in sys.path:
        sys.path.insert(0, _p)

import concourse.bass as bass
import concourse.mybir as mybir
import concourse.tile as tile
from concourse.bass_utils import run_bass_kernel_spmd

F32 = mybir.dt.float32
F32R = mybir.dt.float32r
AF = mybir.ActivationFunctionType
OP = mybir.AluOpType

N_CORES = 8
H = 64    # state dim
HT = 100  # hidden dim
NSTREAM = 2

# schedule-tuning knobs (A/B tested via timing.py / TimelineSim)
EMIT = "step"        # "step" | "eval" — stream interleave granularity
TS_ENGINE = "vector"  # "vector" | "gpsimd" — engine for the tensor_scalar adds
F32R_STATE = True    # keep the h state in f32r (skips per-step rounded copy,
                     # costs ~2x final error: state rounded every step)
Z_BUFS = 2
HIN_BUFS = 2
HB_BUFS = 2
H_BUFS = 3
ZG_BUFS = 3


def _legalize_waits(nc: bass.Bass, max_waits: int = 1) -> int:
    """This container's walrus encodes at most ONE sync-wait per instruction
    (hardware EVENTS struct); Tile can attach several. Hoist excess waits onto
    injected same-engine NoOps placed immediately before the instruction —
    engine streams execute in order, so semantics are preserved."""
    # sems named "<Engine>_<ctx>" are each engine's own tick counter; an
    # engine waiting on its OWN sem at a past tick is trivially satisfied
    # (in-order serial execution), so the wait can be dropped instead of
    # spending a NoOp on it.
    self_sem_prefix = {
        mybir.EngineType.Activation: "Activation_",
        mybir.EngineType.PE: "PE_",
        mybir.EngineType.DVE: "DVE_",
        mybir.EngineType.Pool: "Pool_",
    }
    n_new = 0
    for fn in nc.m.functions:
        for bb in fn.blocks:
            new_list = []
            changed = False
            for ins in bb.instructions:
                si = ins.sync_info
                waits = list(si.on_wait) if si and si.on_wait else []
                pref = self_sem_prefix.get(ins.engine)
                if pref is not None and any(
                    (w.ant_name or "").startswith(pref) for w in waits
                ):
                    waits = [w for w in waits
                             if not (w.ant_name or "").startswith(pref)]
                    ins.sync_info = mybir.SyncInfo(
                        on_wait=list(waits),
                        on_update=list(si.on_update) if si.on_update else [],
                    )
                    changed = True
                    si = ins.sync_info
                if len(waits) > max_waits:
                    keep = waits[-max_waits:]
                    for w in waits[:-max_waits]:
                        nop = mybir.InstNoOp(name=f"I-waitsplit-{n_new}")
                        n_new += 1
                        nop.engine = ins.engine
                        nop.sync_info = mybir.SyncInfo(on_wait=[w], on_update=[])
                        new_list.append(nop)
                    ins.sync_info = mybir.SyncInfo(
                        on_wait=keep,
                        on_update=list(si.on_update) if si.on_update else [],
                    )
                    changed = True
                new_list.append(ins)
            if changed:
                bb.instructions = new_list
    return n_new


def build_program(dts: np.ndarray, b_local: int, mm_fast: bool = True,
                  reps: int = 1, timing_mode: bool = False) -> bass.Bass:
    """Build the per-core Bass program. Same program runs on all 8 cores
    (pure data parallel, no collectives). reps>1 repeats the whole
    integration (identical output) — used only for wall-clock timing.
    timing_mode shrinks the output buffer to [2,H,b_local] (every step
    overwrites row 1) so wall-clock isn't dominated by output transfer."""
    n_steps = len(dts)
    T = 2 if timing_mode else n_steps + 1
    cw = b_local // NSTREAM  # stream width (256)

    nc = bass.Bass(trn_type="TRN2", target_bir_lowering=False, debug=False)

    h0t = nc.dram_tensor("h0t", [H, b_local], F32, kind="ExternalInput").ap()
    w1 = nc.dram_tensor("w1", [H, HT], F32, kind="ExternalInput").ap()
    w2 = nc.dram_tensor("w2", [HT, HT], F32, kind="ExternalInput").ap()
    w3 = nc.dram_tensor("w3", [HT, H], F32, kind="ExternalInput").ap()
    w3x2 = nc.dram_tensor("w3x2", [HT, H], F32, kind="ExternalInput").ap()
    b1d = nc.dram_tensor("b1c", [HT, 1], F32, kind="ExternalInput").ap()
    b2d = nc.dram_tensor("b2c", [HT, 1], F32, kind="ExternalInput").ap()
    tab05 = nc.dram_tensor("tab05", [H, n_steps], F32, kind="ExternalInput").ap()
    tabd = nc.dram_tensor("tabd", [H, n_steps], F32, kind="ExternalInput").ap()
    out = nc.dram_tensor("out", [T, H, b_local], F32, kind="ExternalOutput").ap()

    MMDT = F32R if mm_fast else F32  # dtype of matmul operand tiles

    with tile.TileContext(nc) as tc:
        with (
            tc.tile_pool(name="const", bufs=1) as cp,
            tc.tile_pool(name="sb", bufs=1) as sb,
            tc.tile_pool(name="ps", bufs=1, space="PSUM") as ps,
        ):
            W1t = cp.tile([H, HT], MMDT, tag="w1")
            W2t = cp.tile([HT, HT], MMDT, tag="w2")
            W3t = cp.tile([HT, H], MMDT, tag="w3")
            W3x2t = cp.tile([HT, H], MMDT, tag="w3x2")
            b1t = cp.tile([HT, 1], F32, tag="b1")
            b2t = cp.tile([HT, 1], F32, tag="b2")
            t05 = cp.tile([H, n_steps], F32, tag="t05")
            tdt = cp.tile([H, n_steps], F32, tag="tdt")
            for dst, src in (
                (b1t, b1d), (b2t, b2d), (t05, tab05), (tdt, tabd),
            ):
                nc.sync.dma_start(out=dst[:], in_=src)
            # weights: DMA to fp32 staging, then DVE-convert ("round") into
            # the matmul dtype — walrus requires f32r matmul operands to be
            # produced by a rounding-capable compute op, not raw DMA.
            for dst, src, shp, wtag in (
                (W1t, w1, [H, HT], "w1s"), (W2t, w2, [HT, HT], "w2s"),
                (W3t, w3, [HT, H], "w3s"), (W3x2t, w3x2, [HT, H], "w4s"),
            ):
                if mm_fast:
                    stage = sb.tile(shp, F32, tag=wtag, name="wstage")
                    nc.sync.dma_start(out=stage[:], in_=src)
                    nc.vector.tensor_copy(dst[:], stage[:])
                else:
                    nc.sync.dma_start(out=dst[:], in_=src)

            for _rep in range(reps):
                # initial states per stream + t=0 output rows
                state_dt = MMDT if (mm_fast and F32R_STATE) else F32
                h_cur = []
                for s in range(NSTREAM):
                    c0 = s * cw
                    if state_dt is F32:
                        h0s = sb.tile([H, cw], F32, tag=f"h{s}", bufs=H_BUFS,
                                      name="h0s")
                        nc.sync.dma_start(out=h0s[:], in_=h0t[:, c0:c0 + cw])
                        nc.sync.dma_start(out=out[0, :, c0:c0 + cw], in_=h0s[:])
                    else:
                        h0stage = sb.tile([H, cw], F32, tag=f"h0stage{s}",
                                          name="h0stage")
                        nc.sync.dma_start(out=h0stage[:], in_=h0t[:, c0:c0 + cw])
                        nc.sync.dma_start(out=out[0, :, c0:c0 + cw], in_=h0stage[:])
                        h0s = sb.tile([H, cw], state_dt, tag=f"h{s}", bufs=H_BUFS,
                                      name="h0s")
                        nc.vector.tensor_copy(h0s[:], h0stage[:])
                    h_cur.append(h0s)

                ts_eng = nc.gpsimd if TS_ENGINE == "gpsimd" else nc.vector

                def stream_step(s, i):
                    """Emit one stream's RK4 step; yields between evals so
                    the two independent streams can be interleaved in program
                    order (helps the scheduler's greedy priorities)."""
                    dt = np.float32(dts[i])
                    c_half = float(np.float32(0.5) * dt)
                    c_full = float(dt)
                    c_sixth = float(dt / np.float32(6.0))
                    c0 = s * cw
                    h = h_cur[s]

                    # biased base states (b3 folded): h + 0.5*dt*b3, h + dt*b3
                    hb05 = sb.tile([H, cw], F32, tag=f"hb05_{s}", bufs=HB_BUFS,
                                   name="hb05")
                    ts_eng.tensor_scalar_add(hb05[:], h[:], t05[:, i:i + 1])
                    hbd = sb.tile([H, cw], F32, tag=f"hbd_{s}", bufs=HB_BUFS,
                                  name="hbd")
                    ts_eng.tensor_scalar_add(hbd[:], h[:], tdt[:, i:i + 1])

                    if mm_fast and not F32R_STATE:
                        # rounded copy of the fp32 state for eval-1's matmul
                        hin = sb.tile([H, cw], MMDT, tag=f"hr{s}", bufs=HIN_BUFS,
                                      name="hr")
                        nc.vector.tensor_copy(hin[:], h[:])
                    else:
                        hin = h

                    Sp = ps.tile([H, cw], F32, tag=f"S{s}", bufs=1, name="Sp")

                    for e in range(4):
                        # z1 = tanh(W1.T @ hin + b1)
                        z1p = ps.tile([HT, cw], F32, tag=f"zg{s}", bufs=ZG_BUFS,
                                      name="z1p")
                        nc.tensor.matmul(z1p[:], W1t[:], hin[:],
                                         start=True, stop=True)
                        z1s = sb.tile([HT, cw], MMDT, tag=f"z{s}", bufs=Z_BUFS,
                                      name="z1s")
                        nc.scalar.activation(z1s[:], z1p[:], AF.Tanh, bias=b1t[:])
                        # z2 = tanh(W2.T @ z1 + b2)
                        z2p = ps.tile([HT, cw], F32, tag=f"zg{s}", bufs=ZG_BUFS,
                                      name="z2p")
                        nc.tensor.matmul(z2p[:], W2t[:], z1s[:],
                                         start=True, stop=True)
                        z2s = sb.tile([HT, cw], MMDT, tag=f"z{s}", bufs=Z_BUFS,
                                      name="z2s")
                        nc.scalar.activation(z2s[:], z2p[:], AF.Tanh, bias=b2t[:])
                        # g_e = W3.T @ z2 (= k_e - b3); accumulate RK4 sum in Sp
                        # with weights 1,2,2,1 via stationary W3 / 2*W3.
                        w_acc = W3t if e in (0, 3) else W3x2t
                        nc.tensor.matmul(Sp[:], w_acc[:], z2s[:],
                                         start=(e == 0), stop=(e == 3))
                        if e < 3:
                            # g_e also to its own bank, to build eval e+1 input
                            ge = ps.tile([HT, cw], F32, tag=f"zg{s}", bufs=ZG_BUFS,
                                         name="ge")
                            nc.tensor.matmul(ge[:H, :], W3t[:], z2s[:],
                                             start=True, stop=True)
                            # next eval input: base + c*g_e
                            c = c_half if e < 2 else c_full
                            base = hb05 if e < 2 else hbd
                            hin = sb.tile([H, cw], MMDT, tag=f"hin{s}",
                                          bufs=HIN_BUFS, name="hin")
                            nc.vector.scalar_tensor_tensor(
                                hin[:], ge[:H, :], c, base[:], OP.mult, OP.add)
                        yield

                    # h_next = h + dt*b3 + (dt/6) * S
                    hn = sb.tile([H, cw], state_dt, tag=f"h{s}", bufs=H_BUFS,
                                 name="hn")
                    nc.vector.scalar_tensor_tensor(
                        hn[:], Sp[:], c_sixth, hbd[:], OP.mult, OP.add)
                    t_out = 1 if timing_mode else i + 1
                    hn_out = hn[:] if state_dt is F32 else hn[:].bitcast(F32)
                    nc.sync.dma_start(out=out[t_out, :, c0:c0 + cw], in_=hn_out)
                    h_cur[s] = hn
                    yield

                for i in range(n_steps):
                    if EMIT == "eval":
                        gens = [stream_step(s, i) for s in range(NSTREAM)]
                        alive = list(gens)
                        while alive:
                            for g in list(alive):
                                try:
                                    next(g)
                                except StopIteration:
                                    alive.remove(g)
                    else:
                        for s in range(NSTREAM):
                            for _ in stream_step(s, i):
                                pass
    return nc


def kernel(h0, t, W1, b1, W2, b2, W3, b3):
    h0 = np.ascontiguousarray(np.asarray(h0, dtype=np.float32))
    t = np.asarray(t, dtype=np.float32)
    W1 = np.ascontiguousarray(np.asarray(W1, dtype=np.float32))
    b1 = np.asarray(b1, dtype=np.float32)
    W2 = np.ascontiguousarray(np.asarray(W2, dtype=np.float32))
    b2 = np.asarray(b2, dtype=np.float32)
    W3 = np.ascontiguousarray(np.asarray(W3, dtype=np.float32))
    b3 = np.asarray(b3, dtype=np.float32)

    B = h0.shape[0]
    T = t.shape[0]
    b_local = B // N_CORES

    dts = (t[1:] - t[:-1]).astype(np.float32)
    nc = build_program(dts, b_local, mm_fast=MM_FAST)
    _legalize_waits(nc)

    tab05 = np.ascontiguousarray(np.outer(b3, np.float32(0.5) * dts).astype(np.float32))
    tabd = np.ascontiguousarray(np.outer(b3, dts).astype(np.float32))
    w3x2 = (np.float32(2.0) * W3).astype(np.float32)

    common = {
        "w1": W1,
        "w2": W2,
        "w3": W3,
        "w3x2": w3x2,
        "b1c": np.ascontiguousarray(b1.reshape(HT, 1)),
        "b2c": np.ascontiguousarray(b2.reshape(HT, 1)),
        "tab05": tab05,
        "tabd": tabd,
    }
    in_maps = []
    for c in range(N_CORES):
        h0c = np.ascontiguousarray(h0[c * b_local:(c + 1) * b_local].T)
        in_maps.append({**common, "h0t": h0c})

    trace = bool(os.environ.get("KERNEL_TRACE"))
    res = run_bass_kernel_spmd(nc, in_maps, list(range(N_CORES)), trace=trace)
    global LAST_RESULTS
    LAST_RESULTS = res

    full = np.empty((B, T, h0.shape[1]), np.float32)
    for c in range(N_CORES):
        # [T, H, b_local] -> [b_local, T, H]
        full[c * b_local:(c + 1) * b_local] = res.results[c]["out"].transpose(2, 0, 1)
    return full


MM_FAST = True  # float32r matmul fast path (1 cyc/row at N>=256)
LAST_RESULTS = None  # BassKernelResults of the most recent run (for test.py)


# revision 27
# speedup vs baseline: 7.3768x; 1.0149x over previous
"""Trainium2 Bass kernel: RK4 neural-ODE solver (nn_DiffeqSolver).

Reference semantics (see problem): MLP f(h) = tanh(tanh(h@W1+b1)@W2+b2)@W3+b3,
integrated with RK4 over a time grid t (199 steps), returning all states
[B, T, H].

Strategy
--------
- Data-parallel: batch B=4096 split across 8 NeuronCores (512 rows each).
- Feature-major on-chip layout: h is [H=64 (partitions), batch (free)], so each
  MLP matmul is a single TensorE matmul with the (tiny) weight as the
  stationary operand and batch as the moving free dim.
- Each core's 512 rows are split into 2 independent 256-wide "streams" whose
  199-step chains never interact -> Tile pipelines them across TensorE (matmul,
  float32r fast path), ScalarE (tanh+bias) and DVE (RK4 axpy/combine).
- b3 is folded into per-step biased states: h_b05 = h + 0.5*dt*b3 and
  h_bd = h + dt*b3 (precomputed host tables outer(b3, dts)), so the last
  matmul's output g = f(h) - b3 can be consumed directly from PSUM.
- The RK4 sum k1+2k2+2k3+k4 (in g-space) is accumulated in a single PSUM bank
  by 4 matmuls using stationary W3 / 2*W3; combine is ONE DVE op:
  h_next = (dt/6)*S + h_bd.
- Per-step dt immediates are baked at build time from the actual `t` input,
  so non-uniform grids work.
- Output is written time-major [T, H, 512] per core (perfectly contiguous DMA)
  and transposed to [512, T, H] on the host.
"""

import os
import sys

import numpy as np

for _p in ("/opt/trn_rl_repo", "/root/.axon_site/_ro/trn_rl_repo"):
    if os.path.isdir(_p) and _p not in sys.path:
        sys.path.insert(0, _p)

import concourse.bass as bass
import concourse.mybir as mybir
import concourse.tile as tile
from concourse.bass_utils import run_bass_kernel_spmd

F32 = mybir.dt.float32
F32R = mybir.dt.float32r
AF = mybir.ActivationFunctionType
OP = mybir.AluOpType

N_CORES = 8
H = 64    # state dim
HT = 100  # hidden dim
NSTREAM = 2

# schedule-tuning knobs (A/B tested via timing.py / TimelineSim)
EMIT = "step"        # "step" | "eval" — stream interleave granularity
TS_ENGINE = "vector"  # "vector" | "gpsimd" — engine for the tensor_scalar adds
F32R_STATE = False   # keep the h state in f32r (skips per-step rounded copy;
                     # measured 9x worse final error on HW — keep off)
FP32_EVAL1 = True    # eval-1 matmul reads the fp32 state directly as a plain
                     # fp32 matmul (4 cyc/row) instead of a DVE rounded-copy
                     # chain hop + f32r matmul; shorter chain AND exact eval-1
Z_BUFS = 2
HIN_BUFS = 2
HB_BUFS = 2
H_BUFS = 3
ZG_BUFS = 3


def _legalize_waits(nc: bass.Bass, max_waits: int = 1) -> int:
    """This container's walrus encodes at most ONE sync-wait per instruction
    (hardware EVENTS struct); Tile can attach several. Hoist excess waits onto
    injected same-engine NoOps placed immediately before the instruction —
    engine streams execute in order, so semantics are preserved."""
    # sems named "<Engine>_<ctx>" are each engine's own tick counter; an
    # engine waiting on its OWN sem at a past tick is trivially satisfied
    # (in-order serial execution), so the wait can be dropped instead of
    # spending a NoOp on it.
    self_sem_prefix = {
        mybir.EngineType.Activation: "Activation_",
        mybir.EngineType.PE: "PE_",
        mybir.EngineType.DVE: "DVE_",
        mybir.EngineType.Pool: "Pool_",
    }
    n_new = 0
    for fn in nc.m.functions:
        for bb in fn.blocks:
            new_list = []
            changed = False
            for ins in bb.instructions:
                si = ins.sync_info
                waits = list(si.on_wait) if si and si.on_wait else []
                pref = self_sem_prefix.get(ins.engine)
                if pref is not None and any(
                    (w.ant_name or "").startswith(pref) for w in waits
                ):
                    waits = [w for w in waits
                             if not (w.ant_name or "").startswith(pref)]
                    ins.sync_info = mybir.SyncInfo(
                        on_wait=list(waits),
                        on_update=list(si.on_update) if si.on_update else [],
                    )
                    changed = True
                    si = ins.sync_info
                if len(waits) > max_waits:
                    keep = waits[-max_waits:]
                    for w in waits[:-max_waits]:
                        nop = mybir.InstNoOp(name=f"I-waitsplit-{n_new}")
                        n_new += 1
                        nop.engine = ins.engine
                        nop.sync_info = mybir.SyncInfo(on_wait=[w], on_update=[])
                        new_list.append(nop)
                    ins.sync_info = mybir.SyncInfo(
                        on_wait=keep,
                        on_update=list(si.on_update) if si.on_update else [],
                    )
                    changed = True
                new_list.append(ins)
            if changed:
                bb.instructions = new_list
    return n_new


def build_program(dts: np.ndarray, b_local: int, mm_fast: bool = True,
                  reps: int = 1, timing_mode: bool = False) -> bass.Bass:
    """Build the per-core Bass program. Same program runs on all 8 cores
    (pure data parallel, no collectives). reps>1 repeats the whole
    integration (identical output) — used only for wall-clock timing.
    timing_mode shrinks the output buffer to [2,H,b_local] (every step
    overwrites row 1) so wall-clock isn't dominated by output transfer."""
    n_steps = len(dts)
    T = 2 if timing_mode else n_steps + 1
    cw = b_local // NSTREAM  # stream width (256)

    nc = bass.Bass(trn_type="TRN2", target_bir_lowering=False, debug=False)

    h0t = nc.dram_tensor("h0t", [H, b_local], F32, kind="ExternalInput").ap()
    w1 = nc.dram_tensor("w1", [H, HT], F32, kind="ExternalInput").ap()
    w2 = nc.dram_tensor("w2", [HT, HT], F32, kind="ExternalInput").ap()
    w3 = nc.dram_tensor("w3", [HT, H], F32, kind="ExternalInput").ap()
    w3x2 = nc.dram_tensor("w3x2", [HT, H], F32, kind="ExternalInput").ap()
    b1d = nc.dram_tensor("b1c", [HT, 1], F32, kind="ExternalInput").ap()
    b2d = nc.dram_tensor("b2c", [HT, 1], F32, kind="ExternalInput").ap()
    tab05 = nc.dram_tensor("tab05", [H, n_steps], F32, kind="ExternalInput").ap()
    tabd = nc.dram_tensor("tabd", [H, n_steps], F32, kind="ExternalInput").ap()
    out = nc.dram_tensor("out", [T, H, b_local], F32, kind="ExternalOutput").ap()

    MMDT = F32R if mm_fast else F32  # dtype of matmul operand tiles

    with tile.TileContext(nc) as tc:
        with (
            tc.tile_pool(name="const", bufs=1) as cp,
            tc.tile_pool(name="sb", bufs=1) as sb,
            tc.tile_pool(name="ps", bufs=1, space="PSUM") as ps,
        ):
            W1t = cp.tile([H, HT], MMDT, tag="w1")
            W1f = cp.tile([H, HT], F32, tag="w1f")  # fp32 copy for eval-1 mm
            nc.sync.dma_start(out=W1f[:], in_=w1)
            W2t = cp.tile([HT, HT], MMDT, tag="w2")
            W3t = cp.tile([HT, H], MMDT, tag="w3")
            W3x2t = cp.tile([HT, H], MMDT, tag="w3x2")
            b1t = cp.tile([HT, 1], F32, tag="b1")
            b2t = cp.tile([HT, 1], F32, tag="b2")
            t05 = cp.tile([H, n_steps], F32, tag="t05")
            tdt = cp.tile([H, n_steps], F32, tag="tdt")
            for dst, src in (
                (b1t, b1d), (b2t, b2d), (t05, tab05), (tdt, tabd),
            ):
                nc.sync.dma_start(out=dst[:], in_=src)
            # weights: DMA to fp32 staging, then DVE-convert ("round") into
            # the matmul dtype — walrus requires f32r matmul operands to be
            # produced by a rounding-capable compute op, not raw DMA.
            for dst, src, shp, wtag in (
                (W1t, w1, [H, HT], "w1s"), (W2t, w2, [HT, HT], "w2s"),
                (W3t, w3, [HT, H], "w3s"), (W3x2t, w3x2, [HT, H], "w4s"),
            ):
                if mm_fast:
                    stage = sb.tile(shp, F32, tag=wtag, name="wstage")
                    nc.sync.dma_start(out=stage[:], in_=src)
                    nc.vector.tensor_copy(dst[:], stage[:])
                else:
                    nc.sync.dma_start(out=dst[:], in_=src)

            for _rep in range(reps):
                # initial states per stream + t=0 output rows
                state_dt = MMDT if (mm_fast and F32R_STATE) else F32
                h_cur = []
                for s in range(NSTREAM):
                    c0 = s * cw
                    if state_dt is F32:
                        h0s = sb.tile([H, cw], F32, tag=f"h{s}", bufs=H_BUFS,
                                      name="h0s")
                        nc.sync.dma_start(out=h0s[:], in_=h0t[:, c0:c0 + cw])
                        nc.sync.dma_start(out=out[0, :, c0:c0 + cw], in_=h0s[:])
                    else:
                        h0stage = sb.tile([H, cw], F32, tag=f"h0stage{s}",
                                          name="h0stage")
                        nc.sync.dma_start(out=h0stage[:], in_=h0t[:, c0:c0 + cw])
                        nc.sync.dma_start(out=out[0, :, c0:c0 + cw], in_=h0stage[:])
                        h0s = sb.tile([H, cw], state_dt, tag=f"h{s}", bufs=H_BUFS,
                                      name="h0s")
                        nc.vector.tensor_copy(h0s[:], h0stage[:])
                    h_cur.append(h0s)

                ts_eng = nc.gpsimd if TS_ENGINE == "gpsimd" else nc.vector

                def stream_step(s, i):
                    """Emit one stream's RK4 step; yields between evals so
                    the two independent streams can be interleaved in program
                    order (helps the scheduler's greedy priorities)."""
                    dt = np.float32(dts[i])
                    c_half = float(np.float32(0.5) * dt)
                    c_full = float(dt)
                    c_sixth = float(dt / np.float32(6.0))
                    c0 = s * cw
                    h = h_cur[s]

                    # biased base states (b3 folded): h + 0.5*dt*b3, h + dt*b3
                    hb05 = sb.tile([H, cw], F32, tag=f"hb05_{s}", bufs=HB_BUFS,
                                   name="hb05")
                    ts_eng.tensor_scalar_add(hb05[:], h[:], t05[:, i:i + 1])
                    hbd = sb.tile([H, cw], F32, tag=f"hbd_{s}", bufs=HB_BUFS,
                                  name="hbd")
                    ts_eng.tensor_scalar_add(hbd[:], h[:], tdt[:, i:i + 1])

                    if mm_fast and not F32R_STATE and not FP32_EVAL1:
                        # rounded copy of the fp32 state for eval-1's matmul
                        hin = sb.tile([H, cw], MMDT, tag=f"hr{s}", bufs=HIN_BUFS,
                                      name="hr")
                        nc.vector.tensor_copy(hin[:], h[:])
                    else:
                        hin = h

                    Sp = ps.tile([H, cw], F32, tag=f"S{s}", bufs=1, name="Sp")

                    for e in range(4):
                        # z1 = tanh(W1.T @ hin + b1)
                        z1p = ps.tile([HT, cw], F32, tag=f"zg{s}", bufs=ZG_BUFS,
                                      name="z1p")
                        w1_lhs = (W1f if (e == 0 and mm_fast and not F32R_STATE
                                          and FP32_EVAL1) else W1t)
                        nc.tensor.matmul(z1p[:], w1_lhs[:], hin[:],
                                         start=True, stop=True)
                        z1s = sb.tile([HT, cw], MMDT, tag=f"z{s}", bufs=Z_BUFS,
                                      name="z1s")
                        nc.scalar.activation(z1s[:], z1p[:], AF.Tanh, bias=b1t[:])
                        # z2 = tanh(W2.T @ z1 + b2)
                        z2p = ps.tile([HT, cw], F32, tag=f"zg{s}", bufs=ZG_BUFS,
                                      name="z2p")
                        nc.tensor.matmul(z2p[:], W2t[:], z1s[:],
                                         start=True, stop=True)
                        z2s = sb.tile([HT, cw], MMDT, tag=f"z{s}", bufs=Z_BUFS,
                                      name="z2s")
                        nc.scalar.activation(z2s[:], z2p[:], AF.Tanh, bias=b2t[:])
                        # g_e = W3.T @ z2 (= k_e - b3); accumulate RK4 sum in Sp
                        # with weights 1,2,2,1 via stationary W3 / 2*W3.
                        w_acc = W3t if e in (0, 3) else W3x2t
                        nc.tensor.matmul(Sp[:], w_acc[:], z2s[:],
                                         start=(e == 0), stop=(e == 3))
                        if e < 3:
                            # g_e also to its own bank, to build eval e+1 input
                            ge = ps.tile([HT, cw], F32, tag=f"zg{s}", bufs=ZG_BUFS,
                                         name="ge")
                            nc.tensor.matmul(ge[:H, :], W3t[:], z2s[:],
                                             start=True, stop=True)
                            # next eval input: base + c*g_e
                            c = c_half if e < 2 else c_full
                            base = hb05 if e < 2 else hbd
                            hin = sb.tile([H, cw], MMDT, tag=f"hin{s}",
                                          bufs=HIN_BUFS, name="hin")
                            nc.vector.scalar_tensor_tensor(
                                hin[:], ge[:H, :], c, base[:], OP.mult, OP.add)
                        yield

                    # h_next = h + dt*b3 + (dt/6) * S
                    hn = sb.tile([H, cw], state_dt, tag=f"h{s}", bufs=H_BUFS,
                                 name="hn")
                    nc.vector.scalar_tensor_tensor(
                        hn[:], Sp[:], c_sixth, hbd[:], OP.mult, OP.add)
                    t_out = 1 if timing_mode else i + 1
                    hn_out = hn[:] if state_dt is F32 else hn[:].bitcast(F32)
                    nc.sync.dma_start(out=out[t_out, :, c0:c0 + cw], in_=hn_out)
                    h_cur[s] = hn
                    yield

                for i in range(n_steps):
                    if EMIT == "eval":
                        gens = [stream_step(s, i) for s in range(NSTREAM)]
                        alive = list(gens)
                        while alive:
                            for g in list(alive):
                                try:
                                    next(g)
                                except StopIteration:
                                    alive.remove(g)
                    else:
                        for s in range(NSTREAM):
                            for _ in stream_step(s, i):
                                pass
    return nc


def kernel(h0, t, W1, b1, W2, b2, W3, b3):
    h0 = np.ascontiguousarray(np.asarray(h0, dtype=np.float32))
    t = np.asarray(t, dtype=np.float32)
    W1 = np.ascontiguousarray(np.asarray(W1, dtype=np.float32))
    b1 = np.asarray(b1, dtype=np.float32)
    W2 = np.ascontiguousarray(np.asarray(W2, dtype=np.float32))
    b2 = np.asarray(b2, dtype=np.float32)
    W3 = np.ascontiguousarray(np.asarray(W3, dtype=np.float32))
    b3 = np.asarray(b3, dtype=np.float32)

    B = h0.shape[0]
    T = t.shape[0]
    b_local = B // N_CORES

    dts = (t[1:] - t[:-1]).astype(np.float32)
    nc = build_program(dts, b_local, mm_fast=MM_FAST)
    _legalize_waits(nc)

    tab05 = np.ascontiguousarray(np.outer(b3, np.float32(0.5) * dts).astype(np.float32))
    tabd = np.ascontiguousarray(np.outer(b3, dts).astype(np.float32))
    w3x2 = (np.float32(2.0) * W3).astype(np.float32)

    common = {
        "w1": W1,
        "w2": W2,
        "w3": W3,
        "w3x2": w3x2,
        "b1c": np.ascontiguousarray(b1.reshape(HT, 1)),
        "b2c": np.ascontiguousarray(b2.reshape(HT, 1)),
        "tab05": tab05,
        "tabd": tabd,
    }
    in_maps = []
    for c in range(N_CORES):
        h0c = np.ascontiguousarray(h0[c * b_local:(c + 1) * b_local].T)
        in_maps.append({**common, "h0t": h0c})

    trace = bool(os.environ.get("KERNEL_TRACE"))
    res = run_bass_kernel_spmd(nc, in_maps, list(range(N_CORES)), trace=trace)
    global LAST_RESULTS
    LAST_RESULTS = res

    full = np.empty((B, T, h0.shape[1]), np.float32)
    for c in range(N_CORES):
        # [T, H, b_local] -> [b_local, T, H]
        full[c * b_local:(c + 1) * b_local] = res.results[c]["out"].transpose(2, 0, 1)
    return full


MM_FAST = True  # float32r matmul fast path (1 cyc/row at N>=256)
LAST_RESULTS = None  # BassKernelResults of the most recent run (for test.py)


# revision 33
# speedup vs baseline: 7.7445x; 1.0498x over previous
"""Trainium2 Bass kernel: RK4 neural-ODE solver (nn_DiffeqSolver).

Reference semantics (see problem): MLP f(h) = tanh(tanh(h@W1+b1)@W2+b2)@W3+b3,
integrated with RK4 over a time grid t (199 steps), returning all states
[B, T, H].

Strategy
--------
- Data-parallel: batch B=4096 split across 8 NeuronCores (512 rows each).
- Feature-major on-chip layout: h is [H=64 (partitions), batch (free)], so each
  MLP matmul is a single TensorE matmul with the (tiny) weight as the
  stationary operand and batch as the moving free dim.
- Each core's 512 rows are split into 2 independent 256-wide "streams" whose
  199-step chains never interact -> Tile pipelines them across TensorE (matmul,
  float32r fast path), ScalarE (tanh+bias) and DVE (RK4 axpy/combine).
- b3 is folded into per-step biased states: h_b05 = h + 0.5*dt*b3 and
  h_bd = h + dt*b3 (precomputed host tables outer(b3, dts)), so the last
  matmul's output g = f(h) - b3 can be consumed directly from PSUM.
- The RK4 sum k1+2k2+2k3+k4 (in g-space) is accumulated in a single PSUM bank
  by 4 matmuls using stationary W3 / 2*W3; combine is ONE DVE op:
  h_next = (dt/6)*S + h_bd.
- Per-step dt immediates are baked at build time from the actual `t` input,
  so non-uniform grids work.
- Output is written time-major [T, H, 512] per core (perfectly contiguous DMA)
  and transposed to [512, T, H] on the host.
"""

import os
import sys

import numpy as np

for _p in ("/opt/trn_rl_repo", "/root/.axon_site/_ro/trn_rl_repo"):
    if os.path.isdir(_p) and _p not in sys.path:
        sys.path.insert(0, _p)

import concourse.bass as bass
import concourse.mybir as mybir
import concourse.tile as tile
from concourse.bass_utils import run_bass_kernel_spmd

F32 = mybir.dt.float32
F32R = mybir.dt.float32r
AF = mybir.ActivationFunctionType
OP = mybir.AluOpType

N_CORES = 8
H = 64    # state dim
HT = 100  # hidden dim
NSTREAM = 2

# schedule-tuning knobs (A/B tested via timing.py / TimelineSim)
EMIT = "step"        # "step" | "eval" — stream interleave granularity
TS_ENGINE = "vector"  # "vector" | "gpsimd" — engine for the tensor_scalar adds
F32R_STATE = False   # keep the h state in f32r (skips per-step rounded copy;
                     # measured 9x worse final error on HW — keep off)
FP32_EVAL1 = True    # eval-1 matmul reads the fp32 state directly as a plain
                     # fp32 matmul (4 cyc/row) instead of a DVE rounded-copy
                     # chain hop + f32r matmul; shorter chain AND exact eval-1
DUAL_COMBINE = True   # emit the RK4 combine twice: f32r copy on the critical
                      # chain (feeds a fast f32r eval-1 matmul) + fp32 copy
                      # off-chain for the state/output
Z_BUFS = 2
HIN_BUFS = 2
HB_BUFS = 2
H_BUFS = 3
ZG_BUFS = 3


def _legalize_waits(nc: bass.Bass, max_waits: int = 1) -> int:
    """This container's walrus encodes at most ONE sync-wait per instruction
    (hardware EVENTS struct); Tile can attach several. Hoist excess waits onto
    injected same-engine NoOps placed immediately before the instruction —
    engine streams execute in order, so semantics are preserved."""
    # sems named "<Engine>_<ctx>" are each engine's own tick counter; an
    # engine waiting on its OWN sem at a past tick is trivially satisfied
    # (in-order serial execution), so the wait can be dropped instead of
    # spending a NoOp on it.
    self_sem_prefix = {
        mybir.EngineType.Activation: "Activation_",
        mybir.EngineType.PE: "PE_",
        mybir.EngineType.DVE: "DVE_",
        mybir.EngineType.Pool: "Pool_",
    }
    n_new = 0
    for fn in nc.m.functions:
        for bb in fn.blocks:
            new_list = []
            changed = False
            for ins in bb.instructions:
                si = ins.sync_info
                waits = list(si.on_wait) if si and si.on_wait else []
                pref = self_sem_prefix.get(ins.engine)
                if pref is not None and any(
                    (w.ant_name or "").startswith(pref) for w in waits
                ):
                    waits = [w for w in waits
                             if not (w.ant_name or "").startswith(pref)]
                    ins.sync_info = mybir.SyncInfo(
                        on_wait=list(waits),
                        on_update=list(si.on_update) if si.on_update else [],
                    )
                    changed = True
                    si = ins.sync_info
                if len(waits) > max_waits:
                    keep = waits[-max_waits:]
                    for w in waits[:-max_waits]:
                        nop = mybir.InstNoOp(name=f"I-waitsplit-{n_new}")
                        n_new += 1
                        nop.engine = ins.engine
                        nop.sync_info = mybir.SyncInfo(on_wait=[w], on_update=[])
                        new_list.append(nop)
                    ins.sync_info = mybir.SyncInfo(
                        on_wait=keep,
                        on_update=list(si.on_update) if si.on_update else [],
                    )
                    changed = True
                new_list.append(ins)
            if changed:
                bb.instructions = new_list
    return n_new


def build_program(dts: np.ndarray, b_local: int, mm_fast: bool = True,
                  reps: int = 1, timing_mode: bool = False) -> bass.Bass:
    """Build the per-core Bass program. Same program runs on all 8 cores
    (pure data parallel, no collectives). reps>1 repeats the whole
    integration (identical output) — used only for wall-clock timing.
    timing_mode shrinks the output buffer to [2,H,b_local] (every step
    overwrites row 1) so wall-clock isn't dominated by output transfer."""
    n_steps = len(dts)
    T = 2 if timing_mode else n_steps + 1
    cw = b_local // NSTREAM  # stream width (256)

    nc = bass.Bass(trn_type="TRN2", target_bir_lowering=False, debug=False)

    h0t = nc.dram_tensor("h0t", [H, b_local], F32, kind="ExternalInput").ap()
    w1 = nc.dram_tensor("w1", [H, HT], F32, kind="ExternalInput").ap()
    w2 = nc.dram_tensor("w2", [HT, HT], F32, kind="ExternalInput").ap()
    w3 = nc.dram_tensor("w3", [HT, H], F32, kind="ExternalInput").ap()
    w3x2 = nc.dram_tensor("w3x2", [HT, H], F32, kind="ExternalInput").ap()
    b1d = nc.dram_tensor("b1c", [HT, 1], F32, kind="ExternalInput").ap()
    b2d = nc.dram_tensor("b2c", [HT, 1], F32, kind="ExternalInput").ap()
    tab05 = nc.dram_tensor("tab05", [H, n_steps], F32, kind="ExternalInput").ap()
    tabd = nc.dram_tensor("tabd", [H, n_steps], F32, kind="ExternalInput").ap()
    out = nc.dram_tensor("out", [T, H, b_local], F32, kind="ExternalOutput").ap()

    MMDT = F32R if mm_fast else F32  # dtype of matmul operand tiles

    with tile.TileContext(nc) as tc:
        with (
            tc.tile_pool(name="const", bufs=1) as cp,
            tc.tile_pool(name="sb", bufs=1) as sb,
            tc.tile_pool(name="ps", bufs=1, space="PSUM") as ps,
        ):
            W1t = cp.tile([H, HT], MMDT, tag="w1")
            W1f = cp.tile([H, HT], F32, tag="w1f")  # fp32 copy for eval-1 mm
            nc.sync.dma_start(out=W1f[:], in_=w1)
            W2t = cp.tile([HT, HT], MMDT, tag="w2")
            W3t = cp.tile([HT, H], MMDT, tag="w3")
            W3x2t = cp.tile([HT, H], MMDT, tag="w3x2")
            b1t = cp.tile([HT, 1], F32, tag="b1")
            b2t = cp.tile([HT, 1], F32, tag="b2")
            t05 = cp.tile([H, n_steps], F32, tag="t05")
            tdt = cp.tile([H, n_steps], F32, tag="tdt")
            for dst, src in (
                (b1t, b1d), (b2t, b2d), (t05, tab05), (tdt, tabd),
            ):
                nc.sync.dma_start(out=dst[:], in_=src)
            # weights: DMA to fp32 staging, then DVE-convert ("round") into
            # the matmul dtype — walrus requires f32r matmul operands to be
            # produced by a rounding-capable compute op, not raw DMA.
            for dst, src, shp, wtag in (
                (W1t, w1, [H, HT], "w1s"), (W2t, w2, [HT, HT], "w2s"),
                (W3t, w3, [HT, H], "w3s"), (W3x2t, w3x2, [HT, H], "w4s"),
            ):
                if mm_fast:
                    stage = sb.tile(shp, F32, tag=wtag, name="wstage")
                    nc.sync.dma_start(out=stage[:], in_=src)
                    nc.vector.tensor_copy(dst[:], stage[:])
                else:
                    nc.sync.dma_start(out=dst[:], in_=src)

            for _rep in range(reps):
                # initial states per stream + t=0 output rows
                state_dt = MMDT if (mm_fast and F32R_STATE) else F32
                h_cur = []
                for s in range(NSTREAM):
                    c0 = s * cw
                    if state_dt is F32:
                        h0s = sb.tile([H, cw], F32, tag=f"h{s}", bufs=H_BUFS,
                                      name="h0s")
                        nc.sync.dma_start(out=h0s[:], in_=h0t[:, c0:c0 + cw])
                        nc.sync.dma_start(out=out[0, :, c0:c0 + cw], in_=h0s[:])
                    else:
                        h0stage = sb.tile([H, cw], F32, tag=f"h0stage{s}",
                                          name="h0stage")
                        nc.sync.dma_start(out=h0stage[:], in_=h0t[:, c0:c0 + cw])
                        nc.sync.dma_start(out=out[0, :, c0:c0 + cw], in_=h0stage[:])
                        h0s = sb.tile([H, cw], state_dt, tag=f"h{s}", bufs=H_BUFS,
                                      name="h0s")
                        nc.vector.tensor_copy(h0s[:], h0stage[:])
                    h_cur.append(h0s)

                ts_eng = nc.gpsimd if TS_ENGINE == "gpsimd" else nc.vector
                h_mm = [None] * NSTREAM

                def stream_step(s, i):
                    """Emit one stream's RK4 step; yields between evals so
                    the two independent streams can be interleaved in program
                    order (helps the scheduler's greedy priorities)."""
                    dt = np.float32(dts[i])
                    c_half = float(np.float32(0.5) * dt)
                    c_full = float(dt)
                    c_sixth = float(dt / np.float32(6.0))
                    c0 = s * cw
                    h = h_cur[s]

                    # biased base states (b3 folded): h + 0.5*dt*b3, h + dt*b3
                    hb05 = sb.tile([H, cw], F32, tag=f"hb05_{s}", bufs=HB_BUFS,
                                   name="hb05")
                    ts_eng.tensor_scalar_add(hb05[:], h[:], t05[:, i:i + 1])
                    hbd = sb.tile([H, cw], F32, tag=f"hbd_{s}", bufs=HB_BUFS,
                                  name="hbd")
                    ts_eng.tensor_scalar_add(hbd[:], h[:], tdt[:, i:i + 1])

                    if mm_fast and not F32R_STATE and DUAL_COMBINE and i > 0:
                        hin = h_mm[s]  # f32r twin of h from the dual combine
                    elif mm_fast and not F32R_STATE and not FP32_EVAL1:
                        # rounded copy of the fp32 state for eval-1's matmul
                        hin = sb.tile([H, cw], MMDT, tag=f"hr{s}", bufs=HIN_BUFS,
                                      name="hr")
                        nc.vector.tensor_copy(hin[:], h[:])
                    else:
                        hin = h

                    Sp = ps.tile([H, cw], F32, tag=f"S{s}", bufs=1, name="Sp")

                    for e in range(4):
                        # z1 = tanh(W1.T @ hin + b1)
                        z1p = ps.tile([HT, cw], F32, tag=f"zg{s}", bufs=ZG_BUFS,
                                      name="z1p")
                        use_fp32_m1 = (e == 0 and mm_fast and not F32R_STATE
                                       and FP32_EVAL1
                                       and not (DUAL_COMBINE and i > 0))
                        w1_lhs = W1f if use_fp32_m1 else W1t
                        nc.tensor.matmul(z1p[:], w1_lhs[:], hin[:],
                                         start=True, stop=True)
                        z1s = sb.tile([HT, cw], MMDT, tag=f"z{s}", bufs=Z_BUFS,
                                      name="z1s")
                        nc.scalar.activation(z1s[:], z1p[:], AF.Tanh, bias=b1t[:])
                        # z2 = tanh(W2.T @ z1 + b2)
                        z2p = ps.tile([HT, cw], F32, tag=f"zg{s}", bufs=ZG_BUFS,
                                      name="z2p")
                        nc.tensor.matmul(z2p[:], W2t[:], z1s[:],
                                         start=True, stop=True)
                        z2s = sb.tile([HT, cw], MMDT, tag=f"z{s}", bufs=Z_BUFS,
                                      name="z2s")
                        nc.scalar.activation(z2s[:], z2p[:], AF.Tanh, bias=b2t[:])
                        # g_e = W3.T @ z2 (= k_e - b3); accumulate RK4 sum in Sp
                        # with weights 1,2,2,1 via stationary W3 / 2*W3.
                        w_acc = W3t if e in (0, 3) else W3x2t
                        nc.tensor.matmul(Sp[:], w_acc[:], z2s[:],
                                         start=(e == 0), stop=(e == 3))
                        if e < 3:
                            # g_e also to its own bank, to build eval e+1 input
                            ge = ps.tile([HT, cw], F32, tag=f"zg{s}", bufs=ZG_BUFS,
                                         name="ge")
                            nc.tensor.matmul(ge[:H, :], W3t[:], z2s[:],
                                             start=True, stop=True)
                            # next eval input: base + c*g_e
                            c = c_half if e < 2 else c_full
                            base = hb05 if e < 2 else hbd
                            hin = sb.tile([H, cw], MMDT, tag=f"hin{s}",
                                          bufs=HIN_BUFS, name="hin")
                            nc.vector.scalar_tensor_tensor(
                                hin[:], ge[:H, :], c, base[:], OP.mult, OP.add)
                        yield

                    # h_next = h + dt*b3 + (dt/6) * S
                    if mm_fast and not F32R_STATE and DUAL_COMBINE:
                        hmm = sb.tile([H, cw], MMDT, tag=f"hmm{s}", bufs=2,
                                      name="hmm")
                        nc.vector.scalar_tensor_tensor(
                            hmm[:], Sp[:], c_sixth, hbd[:], OP.mult, OP.add)
                        h_mm[s] = hmm
                    hn = sb.tile([H, cw], state_dt, tag=f"h{s}", bufs=H_BUFS,
                                 name="hn")
                    nc.vector.scalar_tensor_tensor(
                        hn[:], Sp[:], c_sixth, hbd[:], OP.mult, OP.add)
                    t_out = 1 if timing_mode else i + 1
                    hn_out = hn[:] if state_dt is F32 else hn[:].bitcast(F32)
                    nc.sync.dma_start(out=out[t_out, :, c0:c0 + cw], in_=hn_out)
                    h_cur[s] = hn
                    yield

                for i in range(n_steps):
                    if EMIT == "eval":
                        gens = [stream_step(s, i) for s in range(NSTREAM)]
                        alive = list(gens)
                        while alive:
                            for g in list(alive):
                                try:
                                    next(g)
                                except StopIteration:
                                    alive.remove(g)
                    else:
                        for s in range(NSTREAM):
                            for _ in stream_step(s, i):
                                pass
    return nc


def kernel(h0, t, W1, b1, W2, b2, W3, b3):
    h0 = np.ascontiguousarray(np.asarray(h0, dtype=np.float32))
    t = np.asarray(t, dtype=np.float32)
    W1 = np.ascontiguousarray(np.asarray(W1, dtype=np.float32))
    b1 = np.asarray(b1, dtype=np.float32)
    W2 = np.ascontiguousarray(np.asarray(W2, dtype=np.float32))
    b2 = np.asarray(b2, dtype=np.float32)
    W3 = np.ascontiguousarray(np.asarray(W3, dtype=np.float32))
    b3 = np.asarray(b3, dtype=np.float32)

    B = h0.shape[0]
    T = t.shape[0]
    b_local = B // N_CORES

    dts = (t[1:] - t[:-1]).astype(np.float32)
    nc = build_program(dts, b_local, mm_fast=MM_FAST)
    _legalize_waits(nc)

    tab05 = np.ascontiguousarray(np.outer(b3, np.float32(0.5) * dts).astype(np.float32))
    tabd = np.ascontiguousarray(np.outer(b3, dts).astype(np.float32))
    w3x2 = (np.float32(2.0) * W3).astype(np.float32)

    common = {
        "w1": W1,
        "w2": W2,
        "w3": W3,
        "w3x2": w3x2,
        "b1c": np.ascontiguousarray(b1.reshape(HT, 1)),
        "b2c": np.ascontiguousarray(b2.reshape(HT, 1)),
        "tab05": tab05,
        "tabd": tabd,
    }
    in_maps = []
    for c in range(N_CORES):
        h0c = np.ascontiguousarray(h0[c * b_local:(c + 1) * b_local].T)
        in_maps.append({**common, "h0t": h0c})

    trace = bool(os.environ.get("KERNEL_TRACE"))
    res = run_bass_kernel_spmd(nc, in_maps, list(range(N_CORES)), trace=trace)
    global LAST_RESULTS
    LAST_RESULTS = res

    full = np.empty((B, T, h0.shape[1]), np.float32)
    for c in range(N_CORES):
        # [T, H, b_local] -> [b_local, T, H]
        full[c * b_local:(c + 1) * b_local] = res.results[c]["out"].transpose(2, 0, 1)
    return full


MM_FAST = True  # float32r matmul fast path (1 cyc/row at N>=256)
LAST_RESULTS = None  # BassKernelResults of the most recent run (for test.py)
